# revision 36
# baseline (speedup 1.0000x reference)
"""Distributed Trainium2 Bass kernel for AssignmentSimilarityNet (GNN message passing).

Strategy:
  - Shard the track (A=384) dimension across 8 NeuronCores: A_local = 48.
  - On-device layout is feature-major ("transposed"): activations live in SBUF as
    (feat=128 partitions, edge columns) with edge column index = a*B + b (a-major).
  - The edge MLP's first layer (concat of [E, E0, track_embed, curr_embed] @ W_e1)
    is decomposed into block matmuls: per-edge terms (E, E0) are real matmuls,
    the track term folds into a per-partition bias (per a-tile), and the curr term
    is one extra matmul with the shared curr-embedding matrix as rhs.
  - Per step, only the sum-over-A (for the curr-node update) crosses cores: one
    AllReduce of a (128, 384) bf16 partial sum.  The classifier for the current
    step is emitted after the AllReduce is issued so it overlaps the collective.
    The last step needs no node updates at all, so only 3 AllReduces total.
  - The classifier's final (128 -> 1) layer runs transposed (lhsT = hidden
    chunks, rhs = w_c2, N=1) so each step's 48*3 outputs land as columns of one
    (128, 144) PSUM tile, evacuated by a single activation op; the host
    de-transposes.  b_c2 (a scalar) is added on the host after gather.
  - bf16 operand storage / PE compute, f32 PSUM accumulation, f32 outputs.
"""
import numpy as np

import sys
for _p in ('/opt/trn_rl_repo',):
    if _p not in sys.path:
        sys.path.insert(0, _p)

import ml_dtypes

BF16 = ml_dtypes.bfloat16

A = 384
B = 384
NC_N = 8
AL = A // NC_N          # 48 tracks per core
E = 128                 # edge/node dim
RD = 512                # reid dim
S = 4                   # NUM_STEPS
KC = RD // 128          # K chunks for reid matmuls

# order of the (128,128) weight blocks inside the stacked 'wstack' tensor
W_NAMES = ['w_ei2', 'w_sum', 'w_ee', 'w_eie', 'w_ea', 'w_eb', 'w_e2', 'w_c1',
           'w_n1c', 'w_n1s', 'w_n2']
B_NAMES = ['b_cnn', 'b_ei1', 'b_ei2', 'b_e1', 'b_e2', 'b_c1', 'b_n1', 'b_n2']


def _build_nc():
    import concourse.bass as bass
    import concourse.tile as tile
    from concourse import bacc, mybir

    f32 = mybir.dt.float32
    b16 = mybir.dt.bfloat16
    AF = mybir.ActivationFunctionType
    OP = mybir.AluOpType
    RG = [list(range(NC_N))]

    nc = bacc.Bacc(None, target_bir_lowering=False, debug=False)

    def din(name, shape, dt=b16):
        return nc.declare_dram_parameter(name, list(shape), dt, isOutput=False)

    geo_d = din('geo', (5, AL * B))
    ta_nt_d = din('ta_nt', (RD, AL))
    ca_nt_d = din('ca_nt', (RD, B))
    tr_rawt_d = din('tr_rawt', (RD, AL))
    ca_rawt_d = din('ca_rawt', (RD, B))
    w_cnn_d = din('w_cnn', (RD, E))
    w_geo_d = din('w_geo', (5, E))
    w_reid_d = din('w_reid', (1, E))
    w_c2_d = din('w_c2', (E, 1))
    wstack_d = din('wstack', (E, len(W_NAMES) * E))
    bstack_d = din('bstack', (E, len(B_NAMES)), f32)

    # transposed output staging: out[r, s, a, j] = cls[s, a, j*128+r]
    out_d = nc.declare_dram_parameter('out', [128, S * AL * 3], f32, isOutput=True)

    with tile.TileContext(nc) as tc:
        with (
            tc.tile_pool(name='const', bufs=1) as cpool,
            tc.tile_pool(name='big', bufs=1) as big,
            tc.tile_pool(name='h1', bufs=3) as h1p,
            tc.tile_pool(name='hc', bufs=3) as hcp,
            tc.tile_pool(name='small', bufs=2) as sp,
            tc.tile_pool(name='psA', bufs=3, space=bass.MemorySpace.PSUM) as psA,
            tc.tile_pool(name='psB', bufs=2, space=bass.MemorySpace.PSUM) as psB,
            tc.tile_pool(name='psC', bufs=2, space=bass.MemorySpace.PSUM) as psC,
            tc.tile_pool(name='psD', bufs=1, space=bass.MemorySpace.PSUM) as psD,
            tc.tile_pool(name='dram', bufs=2, space='DRAM') as dram,
        ):
            # ---- input DMA (issue spread across engines for queue parallelism) ----
            wstack = cpool.tile([E, len(W_NAMES) * E], b16, tag='wstack')
            nc.gpsimd.dma_start(wstack[:], wstack_d[:])
            W = {n: wstack[:, i * E:(i + 1) * E] for i, n in enumerate(W_NAMES)}
            bstack = cpool.tile([E, len(B_NAMES)], f32, tag='bstack')
            nc.gpsimd.dma_start(bstack[:], bstack_d[:])
            Bc = {n: bstack[:, i:i + 1] for i, n in enumerate(B_NAMES)}
            w_geo = cpool.tile([5, E], b16, tag='w_geo')
            nc.gpsimd.dma_start(w_geo[:], w_geo_d[:])
            w_reid = cpool.tile([1, E], b16, tag='w_reid')
            nc.gpsimd.dma_start(w_reid[:], w_reid_d[:])
            w_c2 = cpool.tile([E, 1], b16, tag='w_c2')
            nc.gpsimd.dma_start(w_c2[:], w_c2_d[:])

            # warm up the collective path during the prologue (real AR shape)
            warm_in = dram.tile([E, B], b16, tag='warm_in')
            warm_out = dram.tile([E, B], b16, tag='warm_out')
            nc.gpsimd.dma_start(warm_in[:], ca_rawt_d[0:128, :])
            nc.gpsimd.collective_compute(
                'AllReduce', OP.add, ins=[warm_in[:].opt()],
                outs=[warm_out[:].opt()], replica_groups=RG)

            def load_chunks(dref, ncols, name, eng):
                ts = []
                for k in range(KC):
                    t = cpool.tile([128, ncols], b16, tag=f'{name}{k}')
                    eng.dma_start(t[:], dref[k * 128:(k + 1) * 128, :])
                    ts.append(t)
                return ts

            geo = big.tile([5, AL * B], b16)
            gchunk = AL * B // 4
            for k in range(4):
                nc.scalar.dma_start(
                    geo[:, k * gchunk:(k + 1) * gchunk],
                    geo_d[:, k * gchunk:(k + 1) * gchunk])

            ta_nt = load_chunks(ta_nt_d, AL, 'ta_nt', nc.sync)
            ca_nt = load_chunks(ca_nt_d, B, 'ca_nt', nc.sync)
            tr_rawt = load_chunks(tr_rawt_d, AL, 'tr_rawt', nc.scalar)
            ca_rawt = load_chunks(ca_rawt_d, B, 'ca_rawt', nc.scalar)
            w_cnn = load_chunks(w_cnn_d, E, 'w_cnn', nc.sync)

            E0 = big.tile([E, AL * B], b16)
            Ecur = big.tile([E, AL * B], b16)
            ostage = big.tile([128, S * AL * 3], f32)

            # ---- prologue compute ----
            # dist (48, B) = ta_n @ ca_n.T   (cosine similarity; sign folded into
            # w_reid) -- first, since the E0 chain depends on it
            dist = sp.tile([AL, B], b16, tag='dist')
            p = psA.tile([E, B], f32, tag='pA')
            for k in range(KC):
                nc.tensor.matmul(p[:AL, :], ta_nt[k][:], ca_nt[k][:],
                                 start=(k == 0), stop=(k == KC - 1))
            nc.scalar.activation(dist[:], p[:AL, :], AF.Copy)
            # flatten to one partition so per-a slices are legal matmul rhs
            dist_row = big.tile([1, AL * B], b16)
            dr3 = dist_row[:].rearrange('o (a x) -> o a x', a=AL)
            q = AL // 4
            for k in range(4):
                nc.sync.dma_start(dr3[:, k * q:(k + 1) * q, :],
                                  dist[k * q:(k + 1) * q, :])

            # current embeds CU0 = relu(W_cnn.T @ ca_raw.T + b_cnn)  (128, B)
            CU = sp.tile([E, B], b16, tag='CU')
            p = psA.tile([E, B], f32, tag='pA')
            for k in range(KC):
                nc.tensor.matmul(p[:], w_cnn[k][:], ca_rawt[k][:],
                                 start=(k == 0), stop=(k == KC - 1))
            nc.scalar.activation(CU[:], p[:], AF.Relu, bias=Bc['b_cnn'])

            # track embeds TR0 (128, AL)
            TR = sp.tile([E, AL], b16, tag='TR')
            p = psB.tile([E, B], f32, tag='pB')
            for k in range(KC):
                nc.tensor.matmul(p[:, :AL], w_cnn[k][:], tr_rawt[k][:],
                                 start=(k == 0), stop=(k == KC - 1))
            nc.scalar.activation(TR[:], p[:, :AL], AF.Relu, bias=Bc['b_cnn'])

            # abias0 = W_ea.T @ TR0 + b_e1   (128, AL) f32
            abias = sp.tile([E, AL], f32, tag='abias')
            p = psB.tile([E, B], f32, tag='pB')
            nc.tensor.matmul(p[:, :AL], W['w_ea'], TR[:])
            nc.scalar.activation(abias[:], p[:, :AL], AF.Identity, bias=Bc['b_e1'])

            # ---- initial edge embeddings E0 ----
            for a in range(AL):
                sl = slice(a * B, (a + 1) * B)
                p1 = psA.tile([E, B], f32, tag='pA')
                nc.tensor.matmul(p1[:], w_geo[:], geo[:, sl], start=True, stop=False)
                nc.tensor.matmul(p1[:], w_reid[:], dist_row[:, sl],
                                 start=False, stop=True)
                h1 = h1p.tile([E, B], b16)
                if a % 2 == 0:
                    nc.vector.tensor_scalar(h1[:], p1[:], Bc['b_ei1'], 0.0,
                                            OP.add, OP.max)
                else:
                    nc.scalar.activation(h1[:], p1[:], AF.Relu, bias=Bc['b_ei1'])
                p2 = psB.tile([E, B], f32, tag='pB')
                nc.tensor.matmul(p2[:], W['w_ei2'], h1[:])
                if a % 2 == 0:
                    nc.scalar.activation(E0[:, sl], p2[:], AF.Relu, bias=Bc['b_ei2'])
                else:
                    nc.vector.tensor_scalar(E0[:, sl], p2[:], Bc['b_ei2'], 0.0,
                                            OP.add, OP.max)

            # resync cores mid-prologue so they reach the first real
            # AllReduce with less skew (its duration includes straggler wait)
            warm2_in = dram.tile([E, 1], f32, tag='warm2_in')
            warm2_out = dram.tile([E, 1], f32, tag='warm2_out')
            nc.gpsimd.dma_start(warm2_in[:], bstack_d[:, 0:1])
            nc.gpsimd.collective_compute(
                'AllReduce', OP.add, ins=[warm2_in[:].opt()],
                outs=[warm2_out[:].opt()], replica_groups=RG)

            # ---- message passing steps ----
            LAG = 2       # classifier trails the edge MLP by this many tiles
            for s in range(S):
                last = (s == S - 1)
                # tiles of classifier work kept back to cover the AllReduce
                DEFER = 0 if last else 40
                Esrc = E0 if s == 0 else Ecur
                if not last:
                    sumb = sp.tile([E, B], b16, tag='sumb')
                    sumA = sp.tile([E, AL], f32, tag='sumA')
                pc2 = psD.tile([128, AL * 3], f32, tag='pD')

                def cls_tile(a):
                    sl = slice(a * B, (a + 1) * B)
                    p3 = psC.tile([E, B], f32, tag='pC')
                    nc.tensor.matmul(p3[:], W['w_c1'], Ecur[:, sl])
                    hc = hcp.tile([E, B], b16)
                    if a % 2 == 0:
                        nc.vector.tensor_scalar(hc[:], p3[:], Bc['b_c1'], 0.0,
                                                OP.add, OP.max)
                    else:
                        nc.scalar.activation(hc[:], p3[:], AF.Relu, bias=Bc['b_c1'])
                    for j in range(3):
                        nc.tensor.matmul(pc2[:, a * 3 + j:a * 3 + j + 1],
                                         hc[:, j * 128:(j + 1) * 128], w_c2[:])

                h1s = {}

                def l1_tile(a):
                    sl = slice(a * B, (a + 1) * B)
                    p1 = psA.tile([E, B], f32, tag='pA')
                    if s == 0:
                        nc.tensor.matmul(p1[:], W['w_sum'], Esrc[:, sl],
                                         start=True, stop=False)
                    else:
                        nc.tensor.matmul(p1[:], W['w_ee'], Esrc[:, sl],
                                         start=True, stop=False)
                        nc.tensor.matmul(p1[:], W['w_eie'], E0[:, sl],
                                         start=False, stop=False)
                    nc.tensor.matmul(p1[:], W['w_eb'], CU[:], start=False, stop=True)
                    h1 = h1p.tile([E, B], b16)
                    nc.vector.tensor_scalar(h1[:], p1[:], abias[:, a:a + 1], 0.0,
                                            OP.add, OP.max)
                    h1s[a] = h1

                def l2_tile(a):
                    sl = slice(a * B, (a + 1) * B)
                    p2 = psB.tile([E, B], f32, tag='pB')
                    nc.tensor.matmul(p2[:], W['w_e2'], h1s.pop(a)[:])
                    if last:
                        nc.scalar.activation(Ecur[:, sl], p2[:], AF.Relu,
                                             bias=Bc['b_e2'])
                    else:
                        nc.scalar.activation(Ecur[:, sl], p2[:], AF.Relu,
                                             bias=Bc['b_e2'],
                                             accum_out=sumA[:, a:a + 1])
                        if a == 0:
                            nc.gpsimd.tensor_copy(sumb[:], Ecur[:, sl])
                        else:
                            nc.gpsimd.tensor_tensor(sumb[:], sumb[:], Ecur[:, sl],
                                                    OP.add)

                # phase A: software pipeline -- per slot emit l1(i), then the
                # classifier of tile i-2, then l2(i-1); every consumer is a
                # full tile behind its producer so the PE never waits
                for i in range(AL):
                    l1_tile(i)
                    if 2 <= i and i - 2 < AL - DEFER:
                        cls_tile(i - 2)
                    if i >= 1:
                        l2_tile(i - 1)
                l2_tile(AL - 1)

                if not last:
                    # all-reduce the partial sum over A, issued before the
                    # classifier so the collective overlaps it; bounce DMAs
                    # chunked across queues to shorten the serial path
                    cc_in = dram.tile([E, B], b16, tag='cc_in')
                    cc_out = dram.tile([E, B], b16, tag='cc_out')
                    hB = B // 2
                    nc.gpsimd.dma_start(cc_in[:, :hB], sumb[:, :hB])
                    nc.gpsimd.dma_start(cc_in[:, hB:], sumb[:, hB:])
                    nc.gpsimd.collective_compute(
                        'AllReduce', OP.add, ins=[cc_in[:].opt()],
                        outs=[cc_out[:].opt()], replica_groups=RG)
                    SB = sp.tile([E, B], b16, tag='SB')
                    nc.gpsimd.dma_start(SB[:, :hB], cc_out[:, :hB])
                    nc.gpsimd.dma_start(SB[:, hB:], cc_out[:, hB:])

                    # curr-node first layer, CU half (AR-independent)
                    pcu = psB.tile([E, B], f32, tag='pB')
                    nc.tensor.matmul(pcu[:], W['w_n1c'], CU[:],
                                     start=True, stop=False)

                    # track-node update (local, AR-independent)
                    sumA_bf = sp.tile([E, AL], b16, tag='sumAb')
                    nc.vector.tensor_copy(sumA_bf[:], sumA[:])
                    pn = psA.tile([E, B], f32, tag='pA')
                    nc.tensor.matmul(pn[:, :AL], W['w_n1c'], TR[:],
                                     start=True, stop=False)
                    nc.tensor.matmul(pn[:, :AL], W['w_n1s'], sumA_bf[:],
                                     start=False, stop=True)
                    htr = sp.tile([E, AL], b16, tag='htr')
                    nc.vector.tensor_scalar(htr[:], pn[:, :AL], Bc['b_n1'], 0.0,
                                            OP.add, OP.max)
                    pn2 = psA.tile([E, B], f32, tag='pA')
                    nc.tensor.matmul(pn2[:, :AL], W['w_n2'], htr[:])
                    TR = sp.tile([E, AL], b16, tag='TR')
                    nc.scalar.activation(TR[:], pn2[:, :AL], AF.Relu, bias=Bc['b_n2'])
                    pn3 = psA.tile([E, B], f32, tag='pA')
                    nc.tensor.matmul(pn3[:, :AL], W['w_ea'], TR[:])
                    abias = sp.tile([E, AL], f32, tag='abias')
                    nc.scalar.activation(abias[:], pn3[:, :AL], AF.Identity,
                                         bias=Bc['b_e1'])

                # deferred classifier tail (overlaps the AllReduce)
                for a in range(max(0, AL - DEFER - LAG), AL):
                    cls_tile(a)

                # drain this step's classifier outputs
                osl = slice(s * AL * 3, (s + 1) * AL * 3)
                nc.scalar.activation(ostage[:, osl], pc2[:], AF.Copy)
                nc.sync.dma_start(out_d[:, osl], ostage[:, osl])

                if last:
                    break

                # curr-node update (replicated; consumes the AllReduce)
                nc.tensor.matmul(pcu[:], W['w_n1s'], SB[:], start=False, stop=True)
                hcu = sp.tile([E, B], b16, tag='hcu')
                nc.vector.tensor_scalar(hcu[:], pcu[:], Bc['b_n1'], 0.0,
                                        OP.add, OP.max)
                pcu2 = psB.tile([E, B], f32, tag='pB')
                nc.tensor.matmul(pcu2[:], W['w_n2'], hcu[:])
                CU = sp.tile([E, B], b16, tag='CU')
                nc.scalar.activation(CU[:], pcu2[:], AF.Relu, bias=Bc['b_n2'])

    nc.compile()
    return nc


_CACHE = {}


def _get_nc():
    if 'nc' not in _CACHE:
        _CACHE['nc'] = _build_nc()
    return _CACHE['nc']


def _prep_in_maps(inputs):
    f32 = np.float32
    ta = np.asarray(inputs['track_app'], f32)
    ca = np.asarray(inputs['current_app'], f32)
    ta_n = ta / np.linalg.norm(ta, axis=1, keepdims=True)
    ca_n = ca / np.linalg.norm(ca, axis=1, keepdims=True)
    tc_ = np.asarray(inputs['track_coords'], f32)
    cc = np.asarray(inputs['current_coords'], f32)
    tr_x = (tc_[:, 0] + tc_[:, 2]) * .5
    tr_y = (tc_[:, 1] + tc_[:, 3]) * .5
    tr_w = np.abs(tc_[:, 2] - tc_[:, 0])
    tr_h = np.abs(tc_[:, 1] - tc_[:, 3])
    cu_x = (cc[:, 0] + cc[:, 2]) * .5
    cu_y = (cc[:, 1] + cc[:, 3]) * .5
    cu_w = np.abs(cc[:, 2] - cc[:, 0])
    cu_h = np.abs(cc[:, 1] - cc[:, 3])
    den = tr_w[:, None] + cu_w[None, :]
    d_x = 2.0 * (cu_x[None, :] - tr_x[:, None]) / den
    d_y = 2.0 * (cu_y[None, :] - tr_y[:, None]) / den
    d_w = np.log(tr_w[:, None] / cu_w[None, :])
    d_h = np.log(tr_h[:, None] / cu_h[None, :])
    d_t = (np.asarray(inputs['track_t'], f32)[:, None]
           - np.asarray(inputs['curr_t'], f32)[None, :])
    geo = np.stack([d_x, d_y, d_w, d_h, d_t], axis=0)  # (5,A,B)

    W_ei1 = np.asarray(inputs['W_ei1'], f32)
    W_e1 = np.asarray(inputs['W_e1'], f32)
    W_n1 = np.asarray(inputs['W_n1'], f32)

    def col(x):
        return np.ascontiguousarray(np.asarray(x, f32).reshape(E, 1))

    wmats = {
        'w_ei2': np.asarray(inputs['W_ei2'], f32),
        'w_sum': W_e1[0:128] + W_e1[128:256],
        'w_ee': W_e1[0:128],
        'w_eie': W_e1[128:256],
        'w_ea': W_e1[256:384],
        'w_eb': W_e1[384:512],
        'w_e2': np.asarray(inputs['W_e2'], f32),
        'w_c1': np.asarray(inputs['W_c1'], f32),
        'w_n1c': W_n1[:128],
        'w_n1s': W_n1[128:],
        'w_n2': np.asarray(inputs['W_n2'], f32),
    }
    bvecs = {
        'b_cnn': col(inputs['b_cnn']),
        'b_ei1': col(np.asarray(inputs['b_ei1'], f32) + W_ei1[5, :]),
        'b_ei2': col(inputs['b_ei2']),
        'b_e1': col(inputs['b_e1']),
        'b_e2': col(inputs['b_e2']),
        'b_c1': col(inputs['b_c1']),
        'b_n1': col(inputs['b_n1']),
        'b_n2': col(inputs['b_n2']),
    }
    shared = {
        'ca_nt': np.ascontiguousarray(ca_n.T).astype(BF16),
        'ca_rawt': np.ascontiguousarray(ca.T).astype(BF16),
        'w_cnn': np.asarray(inputs['W_cnn'], f32).astype(BF16),
        'w_geo': W_ei1[:5, :].astype(BF16),
        'w_reid': (-W_ei1[5:6, :]).astype(BF16),
        'w_c2': np.asarray(inputs['W_c2'], f32).astype(BF16),
        'wstack': np.concatenate([wmats[n] for n in W_NAMES], axis=1).astype(BF16),
        'bstack': np.concatenate([bvecs[n] for n in B_NAMES], axis=1).astype(f32),
    }
    in_maps = []
    for c in range(NC_N):
        rs = slice(c * AL, (c + 1) * AL)
        m = dict(shared)
        m['geo'] = np.ascontiguousarray(geo[:, rs, :]).reshape(5, AL * B).astype(BF16)
        m['ta_nt'] = np.ascontiguousarray(ta_n[rs].T).astype(BF16)
        m['tr_rawt'] = np.ascontiguousarray(ta[rs].T).astype(BF16)
        in_maps.append(m)
    return in_maps


def _assemble(outs, b_c2):
    # outs: per-core (128, S*AL*3) staging; out[r, s, a, j] = cls[s, a, j*128+r]
    full = []
    for o in outs:
        o = np.asarray(o, np.float32).reshape(128, S, AL, 3)
        full.append(np.transpose(o, (1, 2, 3, 0)).reshape(S, AL, B))
    full = np.concatenate(full, axis=1)  # (S, A, B)
    return (full + np.asarray(b_c2, np.float32)[0]).astype(np.float32)


def kernel(**inputs) -> np.ndarray:
    from concourse.bass_utils import run_bass_kernel_spmd

    in_maps = _prep_in_maps(inputs)
    nc = _get_nc()
    res = run_bass_kernel_spmd(nc, in_maps, core_ids=list(range(NC_N)))
    return _assemble([r['out'] for r in res.results], inputs['b_c2'])


if __name__ == '__main__':
    import reference as R
    inp = {k: np.asarray(v) for k, v in R.setup_inputs().items()}
    out = kernel(**inp)
    exp = np.asarray(R.reference(**inp))
    err = np.linalg.norm(out - exp) / np.linalg.norm(exp)
    print('rel_l2', err)


# revision 37
# speedup vs baseline: 1.0072x; 1.0072x over previous
"""Distributed Trainium2 Bass kernel for AssignmentSimilarityNet (GNN message passing).

Strategy:
  - Shard the track (A=384) dimension across 8 NeuronCores: A_local = 48.
  - On-device layout is feature-major ("transposed"): activations live in SBUF as
    (feat=128 partitions, edge columns) with edge column index = a*B + b (a-major).
  - The edge MLP's first layer (concat of [E, E0, track_embed, curr_embed] @ W_e1)
    is decomposed into block matmuls: per-edge terms (E, E0) are real matmuls,
    the track term folds into a per-partition bias (per a-tile), and the curr term
    is one extra matmul with the shared curr-embedding matrix as rhs.
  - Per step, only the sum-over-A (for the curr-node update) crosses cores: one
    AllReduce of a (128, 384) bf16 partial sum.  The classifier for the current
    step is emitted after the AllReduce is issued so it overlaps the collective.
    The last step needs no node updates at all, so only 3 AllReduces total.
  - The classifier's final (128 -> 1) layer runs transposed (lhsT = hidden
    chunks, rhs = w_c2, N=1) so each step's 48*3 outputs land as columns of one
    (128, 144) PSUM tile, evacuated by a single activation op; the host
    de-transposes.  b_c2 (a scalar) is added on the host after gather.
  - bf16 operand storage / PE compute, f32 PSUM accumulation, f32 outputs.
"""
import numpy as np

import sys
for _p in ('/opt/trn_rl_repo',):
    if _p not in sys.path:
        sys.path.insert(0, _p)

import ml_dtypes

BF16 = ml_dtypes.bfloat16

A = 384
B = 384
NC_N = 8
AL = A // NC_N          # 48 tracks per core
E = 128                 # edge/node dim
RD = 512                # reid dim
S = 4                   # NUM_STEPS
KC = RD // 128          # K chunks for reid matmuls

# order of the (128,128) weight blocks inside the stacked 'wstack' tensor
W_NAMES = ['w_ei2', 'w_sum', 'w_ee', 'w_eie', 'w_ea', 'w_eb', 'w_e2', 'w_c1',
           'w_n1c', 'w_n1s', 'w_n2']
B_NAMES = ['b_cnn', 'b_ei1', 'b_ei2', 'b_e1', 'b_e2', 'b_c1', 'b_n1', 'b_n2']


def _build_nc():
    import concourse.bass as bass
    import concourse.tile as tile
    from concourse import bacc, mybir

    f32 = mybir.dt.float32
    b16 = mybir.dt.bfloat16
    AF = mybir.ActivationFunctionType
    OP = mybir.AluOpType
    RG = [list(range(NC_N))]

    nc = bacc.Bacc(None, target_bir_lowering=False, debug=False)

    def din(name, shape, dt=b16):
        return nc.declare_dram_parameter(name, list(shape), dt, isOutput=False)

    geo_d = din('geo', (5, AL * B))
    ta_nt_d = din('ta_nt', (RD, AL))
    ca_nt_d = din('ca_nt', (RD, B))
    tr_rawt_d = din('tr_rawt', (RD, AL))
    ca_rawt_d = din('ca_rawt', (RD, B))
    w_cnn_d = din('w_cnn', (RD, E))
    w_geo_d = din('w_geo', (5, E))
    w_reid_d = din('w_reid', (1, E))
    w_c2_d = din('w_c2', (E, 1))
    wstack_d = din('wstack', (E, len(W_NAMES) * E))
    bstack_d = din('bstack', (E, len(B_NAMES)), f32)

    # transposed output staging: out[r, s, a, j] = cls[s, a, j*128+r]
    out_d = nc.declare_dram_parameter('out', [128, S * AL * 3], f32, isOutput=True)

    with tile.TileContext(nc) as tc:
        with (
            tc.tile_pool(name='const', bufs=1) as cpool,
            tc.tile_pool(name='big', bufs=1) as big,
            tc.tile_pool(name='h1', bufs=3) as h1p,
            tc.tile_pool(name='hc', bufs=3) as hcp,
            tc.tile_pool(name='small', bufs=2) as sp,
            tc.tile_pool(name='psA', bufs=3, space=bass.MemorySpace.PSUM) as psA,
            tc.tile_pool(name='psB', bufs=2, space=bass.MemorySpace.PSUM) as psB,
            tc.tile_pool(name='psC', bufs=2, space=bass.MemorySpace.PSUM) as psC,
            tc.tile_pool(name='psD', bufs=1, space=bass.MemorySpace.PSUM) as psD,
            tc.tile_pool(name='dram', bufs=2, space='DRAM') as dram,
        ):
            # ---- input DMA (issue spread across engines for queue parallelism) ----
            wstack = cpool.tile([E, len(W_NAMES) * E], b16, tag='wstack')
            nc.gpsimd.dma_start(wstack[:], wstack_d[:])
            W = {n: wstack[:, i * E:(i + 1) * E] for i, n in enumerate(W_NAMES)}
            bstack = cpool.tile([E, len(B_NAMES)], f32, tag='bstack')
            nc.gpsimd.dma_start(bstack[:], bstack_d[:])
            Bc = {n: bstack[:, i:i + 1] for i, n in enumerate(B_NAMES)}
            w_geo = cpool.tile([5, E], b16, tag='w_geo')
            nc.gpsimd.dma_start(w_geo[:], w_geo_d[:])
            w_reid = cpool.tile([1, E], b16, tag='w_reid')
            nc.gpsimd.dma_start(w_reid[:], w_reid_d[:])
            w_c2 = cpool.tile([E, 1], b16, tag='w_c2')
            nc.gpsimd.dma_start(w_c2[:], w_c2_d[:])

            # warm up the collective path during the prologue (real AR shape)
            warm_in = dram.tile([E, B], b16, tag='warm_in')
            warm_out = dram.tile([E, B], b16, tag='warm_out')
            nc.gpsimd.dma_start(warm_in[:], ca_rawt_d[0:128, :])
            nc.gpsimd.collective_compute(
                'AllReduce', OP.add, ins=[warm_in[:].opt()],
                outs=[warm_out[:].opt()], replica_groups=RG)

            def load_chunks(dref, ncols, name, eng):
                ts = []
                for k in range(KC):
                    t = cpool.tile([128, ncols], b16, tag=f'{name}{k}')
                    eng.dma_start(t[:], dref[k * 128:(k + 1) * 128, :])
                    ts.append(t)
                return ts

            geo = big.tile([5, AL * B], b16)
            gchunk = AL * B // 4
            for k in range(4):
                nc.scalar.dma_start(
                    geo[:, k * gchunk:(k + 1) * gchunk],
                    geo_d[:, k * gchunk:(k + 1) * gchunk])

            ta_nt = load_chunks(ta_nt_d, AL, 'ta_nt', nc.sync)
            ca_nt = load_chunks(ca_nt_d, B, 'ca_nt', nc.sync)
            tr_rawt = load_chunks(tr_rawt_d, AL, 'tr_rawt', nc.scalar)
            ca_rawt = load_chunks(ca_rawt_d, B, 'ca_rawt', nc.scalar)
            w_cnn = load_chunks(w_cnn_d, E, 'w_cnn', nc.sync)

            E0 = big.tile([E, AL * B], b16)
            Ecur = big.tile([E, AL * B], b16)
            ostage = big.tile([128, S * AL * 3], f32)

            # ---- prologue compute ----
            # dist (48, B) = ta_n @ ca_n.T   (cosine similarity; sign folded into
            # w_reid) -- first, since the E0 chain depends on it
            dist = sp.tile([AL, B], b16, tag='dist')
            p = psA.tile([E, B], f32, tag='pA')
            for k in range(KC):
                nc.tensor.matmul(p[:AL, :], ta_nt[k][:], ca_nt[k][:],
                                 start=(k == 0), stop=(k == KC - 1))
            nc.scalar.activation(dist[:], p[:AL, :], AF.Copy)
            # flatten to one partition so per-a slices are legal matmul rhs
            dist_row = big.tile([1, AL * B], b16)
            dr3 = dist_row[:].rearrange('o (a x) -> o a x', a=AL)
            q = AL // 4
            for k in range(4):
                nc.sync.dma_start(dr3[:, k * q:(k + 1) * q, :],
                                  dist[k * q:(k + 1) * q, :])

            # current embeds CU0 = relu(W_cnn.T @ ca_raw.T + b_cnn)  (128, B)
            CU = sp.tile([E, B], b16, tag='CU')
            p = psA.tile([E, B], f32, tag='pA')
            for k in range(KC):
                nc.tensor.matmul(p[:], w_cnn[k][:], ca_rawt[k][:],
                                 start=(k == 0), stop=(k == KC - 1))
            nc.scalar.activation(CU[:], p[:], AF.Relu, bias=Bc['b_cnn'])

            # track embeds TR0 (128, AL)
            TR = sp.tile([E, AL], b16, tag='TR')
            p = psB.tile([E, B], f32, tag='pB')
            for k in range(KC):
                nc.tensor.matmul(p[:, :AL], w_cnn[k][:], tr_rawt[k][:],
                                 start=(k == 0), stop=(k == KC - 1))
            nc.scalar.activation(TR[:], p[:, :AL], AF.Relu, bias=Bc['b_cnn'])

            # abias0 = W_ea.T @ TR0 + b_e1   (128, AL) f32
            abias = sp.tile([E, AL], f32, tag='abias')
            p = psB.tile([E, B], f32, tag='pB')
            nc.tensor.matmul(p[:, :AL], W['w_ea'], TR[:])
            nc.scalar.activation(abias[:], p[:, :AL], AF.Identity, bias=Bc['b_e1'])

            # ---- initial edge embeddings E0 ----
            for a in range(AL):
                sl = slice(a * B, (a + 1) * B)
                p1 = psA.tile([E, B], f32, tag='pA')
                nc.tensor.matmul(p1[:], w_geo[:], geo[:, sl], start=True, stop=False)
                nc.tensor.matmul(p1[:], w_reid[:], dist_row[:, sl],
                                 start=False, stop=True)
                h1 = h1p.tile([E, B], b16)
                if a % 2 == 0:
                    nc.vector.tensor_scalar(h1[:], p1[:], Bc['b_ei1'], 0.0,
                                            OP.add, OP.max)
                else:
                    nc.scalar.activation(h1[:], p1[:], AF.Relu, bias=Bc['b_ei1'])
                p2 = psB.tile([E, B], f32, tag='pB')
                nc.tensor.matmul(p2[:], W['w_ei2'], h1[:])
                if a % 2 == 0:
                    nc.scalar.activation(E0[:, sl], p2[:], AF.Relu, bias=Bc['b_ei2'])
                else:
                    nc.vector.tensor_scalar(E0[:, sl], p2[:], Bc['b_ei2'], 0.0,
                                            OP.add, OP.max)

            # ---- message passing steps ----
            LAG = 2       # classifier trails the edge MLP by this many tiles
            for s in range(S):
                last = (s == S - 1)
                # tiles of classifier work kept back to cover the AllReduce
                DEFER = 0 if last else 40
                Esrc = E0 if s == 0 else Ecur
                if not last:
                    sumb = sp.tile([E, B], b16, tag='sumb')
                    sumA = sp.tile([E, AL], f32, tag='sumA')
                pc2 = psD.tile([128, AL * 3], f32, tag='pD')

                def cls_tile(a):
                    sl = slice(a * B, (a + 1) * B)
                    p3 = psC.tile([E, B], f32, tag='pC')
                    nc.tensor.matmul(p3[:], W['w_c1'], Ecur[:, sl])
                    hc = hcp.tile([E, B], b16)
                    if a % 2 == 0:
                        nc.vector.tensor_scalar(hc[:], p3[:], Bc['b_c1'], 0.0,
                                                OP.add, OP.max)
                    else:
                        nc.scalar.activation(hc[:], p3[:], AF.Relu, bias=Bc['b_c1'])
                    for j in range(3):
                        nc.tensor.matmul(pc2[:, a * 3 + j:a * 3 + j + 1],
                                         hc[:, j * 128:(j + 1) * 128], w_c2[:])

                h1s = {}

                def l1_tile(a):
                    sl = slice(a * B, (a + 1) * B)
                    p1 = psA.tile([E, B], f32, tag='pA')
                    if s == 0:
                        nc.tensor.matmul(p1[:], W['w_sum'], Esrc[:, sl],
                                         start=True, stop=False)
                    else:
                        nc.tensor.matmul(p1[:], W['w_ee'], Esrc[:, sl],
                                         start=True, stop=False)
                        nc.tensor.matmul(p1[:], W['w_eie'], E0[:, sl],
                                         start=False, stop=False)
                    nc.tensor.matmul(p1[:], W['w_eb'], CU[:], start=False, stop=True)
                    h1 = h1p.tile([E, B], b16)
                    nc.vector.tensor_scalar(h1[:], p1[:], abias[:, a:a + 1], 0.0,
                                            OP.add, OP.max)
                    h1s[a] = h1

                def l2_tile(a):
                    sl = slice(a * B, (a + 1) * B)
                    p2 = psB.tile([E, B], f32, tag='pB')
                    nc.tensor.matmul(p2[:], W['w_e2'], h1s.pop(a)[:])
                    if last:
                        nc.scalar.activation(Ecur[:, sl], p2[:], AF.Relu,
                                             bias=Bc['b_e2'])
                    else:
                        nc.scalar.activation(Ecur[:, sl], p2[:], AF.Relu,
                                             bias=Bc['b_e2'],
                                             accum_out=sumA[:, a:a + 1])
                        if a == 0:
                            nc.gpsimd.tensor_copy(sumb[:], Ecur[:, sl])
                        else:
                            nc.gpsimd.tensor_tensor(sumb[:], sumb[:], Ecur[:, sl],
                                                    OP.add)

                # phase A: software pipeline -- per slot emit l1(i), then the
                # classifier of tile i-2, then l2(i-1); every consumer is a
                # full tile behind its producer so the PE never waits
                for i in range(AL):
                    l1_tile(i)
                    if 2 <= i and i - 2 < AL - DEFER:
                        cls_tile(i - 2)
                    if i >= 1:
                        l2_tile(i - 1)
                l2_tile(AL - 1)

                if not last:
                    # all-reduce the partial sum over A, issued before the
                    # classifier so the collective overlaps it; bounce DMAs
                    # chunked across queues to shorten the serial path
                    cc_in = dram.tile([E, B], b16, tag='cc_in')
                    cc_out = dram.tile([E, B], b16, tag='cc_out')
                    hB = B // 2
                    nc.gpsimd.dma_start(cc_in[:, :hB], sumb[:, :hB])
                    nc.gpsimd.dma_start(cc_in[:, hB:], sumb[:, hB:])
                    nc.gpsimd.collective_compute(
                        'AllReduce', OP.add, ins=[cc_in[:].opt()],
                        outs=[cc_out[:].opt()], replica_groups=RG)
                    SB = sp.tile([E, B], b16, tag='SB')
                    nc.gpsimd.dma_start(SB[:, :hB], cc_out[:, :hB])
                    nc.gpsimd.dma_start(SB[:, hB:], cc_out[:, hB:])

                    # curr-node first layer, CU half (AR-independent)
                    pcu = psB.tile([E, B], f32, tag='pB')
                    nc.tensor.matmul(pcu[:], W['w_n1c'], CU[:],
                                     start=True, stop=False)

                    # track-node update (local, AR-independent)
                    sumA_bf = sp.tile([E, AL], b16, tag='sumAb')
                    nc.vector.tensor_copy(sumA_bf[:], sumA[:])
                    pn = psA.tile([E, B], f32, tag='pA')
                    nc.tensor.matmul(pn[:, :AL], W['w_n1c'], TR[:],
                                     start=True, stop=False)
                    nc.tensor.matmul(pn[:, :AL], W['w_n1s'], sumA_bf[:],
                                     start=False, stop=True)
                    htr = sp.tile([E, AL], b16, tag='htr')
                    nc.vector.tensor_scalar(htr[:], pn[:, :AL], Bc['b_n1'], 0.0,
                                            OP.add, OP.max)
                    pn2 = psA.tile([E, B], f32, tag='pA')
                    nc.tensor.matmul(pn2[:, :AL], W['w_n2'], htr[:])
                    TR = sp.tile([E, AL], b16, tag='TR')
                    nc.scalar.activation(TR[:], pn2[:, :AL], AF.Relu, bias=Bc['b_n2'])
                    pn3 = psA.tile([E, B], f32, tag='pA')
                    nc.tensor.matmul(pn3[:, :AL], W['w_ea'], TR[:])
                    abias = sp.tile([E, AL], f32, tag='abias')
                    nc.scalar.activation(abias[:], pn3[:, :AL], AF.Identity,
                                         bias=Bc['b_e1'])

                # deferred classifier tail (overlaps the AllReduce)
                for a in range(max(0, AL - DEFER - LAG), AL):
                    cls_tile(a)

                # drain this step's classifier outputs
                osl = slice(s * AL * 3, (s + 1) * AL * 3)
                nc.scalar.activation(ostage[:, osl], pc2[:], AF.Copy)
                nc.sync.dma_start(out_d[:, osl], ostage[:, osl])

                if last:
                    break

                # curr-node update (replicated; consumes the AllReduce)
                nc.tensor.matmul(pcu[:], W['w_n1s'], SB[:], start=False, stop=True)
                hcu = sp.tile([E, B], b16, tag='hcu')
                nc.vector.tensor_scalar(hcu[:], pcu[:], Bc['b_n1'], 0.0,
                                        OP.add, OP.max)
                pcu2 = psB.tile([E, B], f32, tag='pB')
                nc.tensor.matmul(pcu2[:], W['w_n2'], hcu[:])
                CU = sp.tile([E, B], b16, tag='CU')
                nc.scalar.activation(CU[:], pcu2[:], AF.Relu, bias=Bc['b_n2'])

    nc.compile()
    return nc


_CACHE = {}


def _get_nc():
    if 'nc' not in _CACHE:
        _CACHE['nc'] = _build_nc()
    return _CACHE['nc']


def _prep_in_maps(inputs):
    f32 = np.float32
    ta = np.asarray(inputs['track_app'], f32)
    ca = np.asarray(inputs['current_app'], f32)
    ta_n = ta / np.linalg.norm(ta, axis=1, keepdims=True)
    ca_n = ca / np.linalg.norm(ca, axis=1, keepdims=True)
    tc_ = np.asarray(inputs['track_coords'], f32)
    cc = np.asarray(inputs['current_coords'], f32)
    tr_x = (tc_[:, 0] + tc_[:, 2]) * .5
    tr_y = (tc_[:, 1] + tc_[:, 3]) * .5
    tr_w = np.abs(tc_[:, 2] - tc_[:, 0])
    tr_h = np.abs(tc_[:, 1] - tc_[:, 3])
    cu_x = (cc[:, 0] + cc[:, 2]) * .5
    cu_y = (cc[:, 1] + cc[:, 3]) * .5
    cu_w = np.abs(cc[:, 2] - cc[:, 0])
    cu_h = np.abs(cc[:, 1] - cc[:, 3])
    den = tr_w[:, None] + cu_w[None, :]
    d_x = 2.0 * (cu_x[None, :] - tr_x[:, None]) / den
    d_y = 2.0 * (cu_y[None, :] - tr_y[:, None]) / den
    d_w = np.log(tr_w[:, None] / cu_w[None, :])
    d_h = np.log(tr_h[:, None] / cu_h[None, :])
    d_t = (np.asarray(inputs['track_t'], f32)[:, None]
           - np.asarray(inputs['curr_t'], f32)[None, :])
    geo = np.stack([d_x, d_y, d_w, d_h, d_t], axis=0)  # (5,A,B)

    W_ei1 = np.asarray(inputs['W_ei1'], f32)
    W_e1 = np.asarray(inputs['W_e1'], f32)
    W_n1 = np.asarray(inputs['W_n1'], f32)

    def col(x):
        return np.ascontiguousarray(np.asarray(x, f32).reshape(E, 1))

    wmats = {
        'w_ei2': np.asarray(inputs['W_ei2'], f32),
        'w_sum': W_e1[0:128] + W_e1[128:256],
        'w_ee': W_e1[0:128],
        'w_eie': W_e1[128:256],
        'w_ea': W_e1[256:384],
        'w_eb': W_e1[384:512],
        'w_e2': np.asarray(inputs['W_e2'], f32),
        'w_c1': np.asarray(inputs['W_c1'], f32),
        'w_n1c': W_n1[:128],
        'w_n1s': W_n1[128:],
        'w_n2': np.asarray(inputs['W_n2'], f32),
    }
    bvecs = {
        'b_cnn': col(inputs['b_cnn']),
        'b_ei1': col(np.asarray(inputs['b_ei1'], f32) + W_ei1[5, :]),
        'b_ei2': col(inputs['b_ei2']),
        'b_e1': col(inputs['b_e1']),
        'b_e2': col(inputs['b_e2']),
        'b_c1': col(inputs['b_c1']),
        'b_n1': col(inputs['b_n1']),
        'b_n2': col(inputs['b_n2']),
    }
    shared = {
        'ca_nt': np.ascontiguousarray(ca_n.T).astype(BF16),
        'ca_rawt': np.ascontiguousarray(ca.T).astype(BF16),
        'w_cnn': np.asarray(inputs['W_cnn'], f32).astype(BF16),
        'w_geo': W_ei1[:5, :].astype(BF16),
        'w_reid': (-W_ei1[5:6, :]).astype(BF16),
        'w_c2': np.asarray(inputs['W_c2'], f32).astype(BF16),
        'wstack': np.concatenate([wmats[n] for n in W_NAMES], axis=1).astype(BF16),
        'bstack': np.concatenate([bvecs[n] for n in B_NAMES], axis=1).astype(f32),
    }
    in_maps = []
    for c in range(NC_N):
        rs = slice(c * AL, (c + 1) * AL)
        m = dict(shared)
        m['geo'] = np.ascontiguousarray(geo[:, rs, :]).reshape(5, AL * B).astype(BF16)
        m['ta_nt'] = np.ascontiguousarray(ta_n[rs].T).astype(BF16)
        m['tr_rawt'] = np.ascontiguousarray(ta[rs].T).astype(BF16)
        in_maps.append(m)
    return in_maps


def _assemble(outs, b_c2):
    # outs: per-core (128, S*AL*3) staging; out[r, s, a, j] = cls[s, a, j*128+r]
    full = []
    for o in outs:
        o = np.asarray(o, np.float32).reshape(128, S, AL, 3)
        full.append(np.transpose(o, (1, 2, 3, 0)).reshape(S, AL, B))
    full = np.concatenate(full, axis=1)  # (S, A, B)
    return (full + np.asarray(b_c2, np.float32)[0]).astype(np.float32)


def kernel(**inputs) -> np.ndarray:
    from concourse.bass_utils import run_bass_kernel_spmd

    in_maps = _prep_in_maps(inputs)
    nc = _get_nc()
    res = run_bass_kernel_spmd(nc, in_maps, core_ids=list(range(NC_N)))
    return _assemble([r['out'] for r in res.results], inputs['b_c2'])


if __name__ == '__main__':
    import reference as R
    inp = {k: np.asarray(v) for k, v in R.setup_inputs().items()}
    out = kernel(**inp)
    exp = np.asarray(R.reference(**inp))
    err = np.linalg.norm(out - exp) / np.linalg.norm(exp)
    print('rel_l2', err)


# revision 39
# speedup vs baseline: 1.1897x; 1.1812x over previous
"""Distributed Trainium2 Bass kernel for AssignmentSimilarityNet (GNN message passing).

Strategy:
  - Shard the track (A=384) dimension across 8 NeuronCores: A_local = 48.
  - On-device layout is feature-major ("transposed"): activations live in SBUF as
    (feat=128 partitions, edge columns) with edge column index = a*B + b (a-major).
  - The edge MLP's first layer (concat of [E, E0, track_embed, curr_embed] @ W_e1)
    is decomposed into block matmuls: per-edge terms (E, E0) are real matmuls,
    the track term folds into a per-partition bias (per a-tile), and the curr term
    is one extra matmul with the shared curr-embedding matrix as rhs.
  - Per step, only the sum-over-A (for the curr-node update) crosses cores: one
    AllReduce of a (128, 384) bf16 partial sum.  The classifier for the current
    step is emitted after the AllReduce is issued so it overlaps the collective.
    The last step needs no node updates at all, so only 3 AllReduces total.
  - The classifier's final (128 -> 1) layer runs transposed (lhsT = hidden
    chunks, rhs = w_c2, N=1) so each step's 48*3 outputs land as columns of one
    (128, 144) PSUM tile, evacuated by a single activation op; the host
    de-transposes.  b_c2 (a scalar) is added on the host after gather.
  - bf16 operand storage / PE compute, f32 PSUM accumulation, f32 outputs.
"""
import numpy as np

import sys
for _p in ('/opt/trn_rl_repo',):
    if _p not in sys.path:
        sys.path.insert(0, _p)

import ml_dtypes

BF16 = ml_dtypes.bfloat16

A = 384
B = 384
NC_N = 8
AL = A // NC_N          # 48 tracks per core
E = 128                 # edge/node dim
RD = 512                # reid dim
S = 4                   # NUM_STEPS
KC = RD // 128          # K chunks for reid matmuls

# order of the (128,128) weight blocks inside the stacked 'wstack' tensor
W_NAMES = ['w_ei2', 'w_sum', 'w_ee', 'w_eie', 'w_ea', 'w_eb', 'w_e2', 'w_c1',
           'w_n1c', 'w_n1s', 'w_n2']
B_NAMES = ['b_cnn', 'b_ei1', 'b_ei2', 'b_e1', 'b_e2', 'b_c1', 'b_n1', 'b_n2']


def _build_nc():
    import concourse.bass as bass
    import concourse.tile as tile
    from concourse import bacc, mybir

    f32 = mybir.dt.float32
    b16 = mybir.dt.bfloat16
    AF = mybir.ActivationFunctionType
    OP = mybir.AluOpType
    RG = [list(range(NC_N))]

    nc = bacc.Bacc(None, target_bir_lowering=False, debug=False)

    def din(name, shape, dt=b16):
        return nc.declare_dram_parameter(name, list(shape), dt, isOutput=False)

    geo_d = din('geo', (5, AL * B))
    ta_nt_d = din('ta_nt', (RD, AL))
    ca_nt_d = din('ca_nt', (RD, B))
    tr_rawt_d = din('tr_rawt', (RD, AL))
    ca_rawt_d = din('ca_rawt', (RD, B))
    w_cnn_d = din('w_cnn', (RD, E))
    w_geo_d = din('w_geo', (5, E))
    w_reid_d = din('w_reid', (1, E))
    w_c2_d = din('w_c2', (E, 1))
    wstack_d = din('wstack', (E, len(W_NAMES) * E))
    bstack_d = din('bstack', (E, len(B_NAMES)), f32)

    # transposed output staging: out[r, s, a, j] = cls[s, a, j*128+r]
    out_d = nc.declare_dram_parameter('out', [128, S * AL * 3], f32, isOutput=True)

    with tile.TileContext(nc) as tc:
        with (
            tc.tile_pool(name='const', bufs=1) as cpool,
            tc.tile_pool(name='big', bufs=1) as big,
            tc.tile_pool(name='h1', bufs=3) as h1p,
            tc.tile_pool(name='hc', bufs=3) as hcp,
            tc.tile_pool(name='small', bufs=2) as sp,
            tc.tile_pool(name='psA', bufs=3, space=bass.MemorySpace.PSUM) as psA,
            tc.tile_pool(name='psB', bufs=2, space=bass.MemorySpace.PSUM) as psB,
            tc.tile_pool(name='psC', bufs=2, space=bass.MemorySpace.PSUM) as psC,
            tc.tile_pool(name='psD', bufs=1, space=bass.MemorySpace.PSUM) as psD,
            tc.tile_pool(name='dram', bufs=2, space='DRAM') as dram,
        ):
            # ---- input DMA (issue spread across engines for queue parallelism) ----
            wstack = cpool.tile([E, len(W_NAMES) * E], b16, tag='wstack')
            nc.gpsimd.dma_start(wstack[:], wstack_d[:])
            W = {n: wstack[:, i * E:(i + 1) * E] for i, n in enumerate(W_NAMES)}
            bstack = cpool.tile([E, len(B_NAMES)], f32, tag='bstack')
            nc.gpsimd.dma_start(bstack[:], bstack_d[:])
            Bc = {n: bstack[:, i:i + 1] for i, n in enumerate(B_NAMES)}
            w_geo = cpool.tile([5, E], b16, tag='w_geo')
            nc.gpsimd.dma_start(w_geo[:], w_geo_d[:])
            w_reid = cpool.tile([1, E], b16, tag='w_reid')
            nc.gpsimd.dma_start(w_reid[:], w_reid_d[:])
            w_c2 = cpool.tile([E, 1], b16, tag='w_c2')
            nc.gpsimd.dma_start(w_c2[:], w_c2_d[:])

            # warm up the collective path during the prologue (real AR shape)
            warm_in = dram.tile([E, B], b16, tag='warm_in')
            warm_out = dram.tile([E, B], b16, tag='warm_out')
            nc.gpsimd.dma_start(warm_in[:], ca_rawt_d[0:128, :])
            nc.gpsimd.collective_compute(
                'AllReduce', OP.add, ins=[warm_in[:].opt()],
                outs=[warm_out[:].opt()], replica_groups=RG)

            def load_chunks(dref, ncols, name, eng):
                ts = []
                for k in range(KC):
                    t = cpool.tile([128, ncols], b16, tag=f'{name}{k}')
                    eng.dma_start(t[:], dref[k * 128:(k + 1) * 128, :])
                    ts.append(t)
                return ts

            geo = big.tile([5, AL * B], b16)
            gchunk = AL * B // 4
            for k in range(4):
                nc.scalar.dma_start(
                    geo[:, k * gchunk:(k + 1) * gchunk],
                    geo_d[:, k * gchunk:(k + 1) * gchunk])

            ta_nt = load_chunks(ta_nt_d, AL, 'ta_nt', nc.sync)
            ca_nt = load_chunks(ca_nt_d, B, 'ca_nt', nc.sync)
            tr_rawt = load_chunks(tr_rawt_d, AL, 'tr_rawt', nc.scalar)
            ca_rawt = load_chunks(ca_rawt_d, B, 'ca_rawt', nc.scalar)
            w_cnn = load_chunks(w_cnn_d, E, 'w_cnn', nc.sync)

            E0 = big.tile([E, AL * B], b16)
            Ecur = big.tile([E, AL * B], b16)
            ostage = big.tile([128, S * AL * 3], f32)

            # ---- prologue compute ----
            # dist (48, B) = ta_n @ ca_n.T   (cosine similarity; sign folded into
            # w_reid) -- first, since the E0 chain depends on it
            dist = sp.tile([AL, B], b16, tag='dist')
            p = psA.tile([E, B], f32, tag='pA')
            for k in range(KC):
                nc.tensor.matmul(p[:AL, :], ta_nt[k][:], ca_nt[k][:],
                                 start=(k == 0), stop=(k == KC - 1))
            nc.scalar.activation(dist[:], p[:AL, :], AF.Copy)
            # flatten to one partition so per-a slices are legal matmul rhs
            dist_row = big.tile([1, AL * B], b16)
            dr3 = dist_row[:].rearrange('o (a x) -> o a x', a=AL)
            q = AL // 4
            for k in range(4):
                nc.gpsimd.dma_start(dr3[:, k * q:(k + 1) * q, :],
                                    dist[k * q:(k + 1) * q, :])

            # current embeds CU0 = relu(W_cnn.T @ ca_raw.T + b_cnn)  (128, B)
            CU = sp.tile([E, B], b16, tag='CU')
            p = psA.tile([E, B], f32, tag='pA')
            for k in range(KC):
                nc.tensor.matmul(p[:], w_cnn[k][:], ca_rawt[k][:],
                                 start=(k == 0), stop=(k == KC - 1))
            nc.scalar.activation(CU[:], p[:], AF.Relu, bias=Bc['b_cnn'])

            # track embeds TR0 (128, AL)
            TR = sp.tile([E, AL], b16, tag='TR')
            p = psB.tile([E, B], f32, tag='pB')
            for k in range(KC):
                nc.tensor.matmul(p[:, :AL], w_cnn[k][:], tr_rawt[k][:],
                                 start=(k == 0), stop=(k == KC - 1))
            nc.scalar.activation(TR[:], p[:, :AL], AF.Relu, bias=Bc['b_cnn'])

            # abias0 = W_ea.T @ TR0 + b_e1   (128, AL) f32
            abias = sp.tile([E, AL], f32, tag='abias')
            p = psB.tile([E, B], f32, tag='pB')
            nc.tensor.matmul(p[:, :AL], W['w_ea'], TR[:])
            nc.scalar.activation(abias[:], p[:, :AL], AF.Identity, bias=Bc['b_e1'])

            # ---- initial edge embeddings E0 ----
            for a in range(AL):
                sl = slice(a * B, (a + 1) * B)
                p1 = psA.tile([E, B], f32, tag='pA')
                nc.tensor.matmul(p1[:], w_geo[:], geo[:, sl], start=True, stop=False)
                nc.tensor.matmul(p1[:], w_reid[:], dist_row[:, sl],
                                 start=False, stop=True)
                h1 = h1p.tile([E, B], b16)
                if a % 2 == 0:
                    nc.vector.tensor_scalar(h1[:], p1[:], Bc['b_ei1'], 0.0,
                                            OP.add, OP.max)
                else:
                    nc.scalar.activation(h1[:], p1[:], AF.Relu, bias=Bc['b_ei1'])
                p2 = psB.tile([E, B], f32, tag='pB')
                nc.tensor.matmul(p2[:], W['w_ei2'], h1[:])
                if a % 2 == 0:
                    nc.scalar.activation(E0[:, sl], p2[:], AF.Relu, bias=Bc['b_ei2'])
                else:
                    nc.vector.tensor_scalar(E0[:, sl], p2[:], Bc['b_ei2'], 0.0,
                                            OP.add, OP.max)

            # ---- message passing steps ----
            LAG = 3       # classifier trails the edge MLP by this many tiles
            for s in range(S):
                last = (s == S - 1)
                # tiles of classifier work kept back to cover the AllReduce
                DEFER = 0 if last else 40
                Esrc = E0 if s == 0 else Ecur
                if not last:
                    sumb = sp.tile([E, B], b16, tag='sumb')
                    sumA = sp.tile([E, AL], f32, tag='sumA')
                pc2 = psD.tile([128, AL * 3], f32, tag='pD')

                def cls_tile(a):
                    sl = slice(a * B, (a + 1) * B)
                    p3 = psC.tile([E, B], f32, tag='pC')
                    nc.tensor.matmul(p3[:], W['w_c1'], Ecur[:, sl])
                    hc = hcp.tile([E, B], b16)
                    if a % 2 == 0:
                        nc.vector.tensor_scalar(hc[:], p3[:], Bc['b_c1'], 0.0,
                                                OP.add, OP.max)
                    else:
                        nc.scalar.activation(hc[:], p3[:], AF.Relu, bias=Bc['b_c1'])
                    for j in range(3):
                        nc.tensor.matmul(pc2[:, a * 3 + j:a * 3 + j + 1],
                                         hc[:, j * 128:(j + 1) * 128], w_c2[:])

                h1s = {}

                def l1_tile(a):
                    sl = slice(a * B, (a + 1) * B)
                    p1 = psA.tile([E, B], f32, tag='pA')
                    if s == 0:
                        nc.tensor.matmul(p1[:], W['w_sum'], Esrc[:, sl],
                                         start=True, stop=False)
                    else:
                        nc.tensor.matmul(p1[:], W['w_ee'], Esrc[:, sl],
                                         start=True, stop=False)
                        nc.tensor.matmul(p1[:], W['w_eie'], E0[:, sl],
                                         start=False, stop=False)
                    nc.tensor.matmul(p1[:], W['w_eb'], CU[:], start=False, stop=True)
                    h1 = h1p.tile([E, B], b16)
                    nc.vector.tensor_scalar(h1[:], p1[:], abias[:, a:a + 1], 0.0,
                                            OP.add, OP.max)
                    h1s[a] = h1

                def l2_tile(a):
                    sl = slice(a * B, (a + 1) * B)
                    p2 = psB.tile([E, B], f32, tag='pB')
                    nc.tensor.matmul(p2[:], W['w_e2'], h1s.pop(a)[:])
                    if last:
                        nc.scalar.activation(Ecur[:, sl], p2[:], AF.Relu,
                                             bias=Bc['b_e2'])
                    else:
                        nc.scalar.activation(Ecur[:, sl], p2[:], AF.Relu,
                                             bias=Bc['b_e2'],
                                             accum_out=sumA[:, a:a + 1])
                        if a == 0:
                            nc.gpsimd.tensor_copy(sumb[:], Ecur[:, sl])
                        else:
                            nc.gpsimd.tensor_tensor(sumb[:], sumb[:], Ecur[:, sl],
                                                    OP.add)

                # phase A: software pipeline -- per slot emit l1(i), then the
                # classifier of tile i-2, then l2(i-1); every consumer is a
                # full tile behind its producer so the PE never waits
                for i in range(AL):
                    l1_tile(i)
                    if LAG <= i and i - LAG < AL - DEFER:
                        cls_tile(i - LAG)
                    if i >= 1:
                        l2_tile(i - 1)
                l2_tile(AL - 1)

                if not last:
                    # all-reduce the partial sum over A, issued before the
                    # classifier so the collective overlaps it; bounce DMAs
                    # chunked across queues to shorten the serial path
                    cc_in = dram.tile([E, B], b16, tag='cc_in')
                    cc_out = dram.tile([E, B], b16, tag='cc_out')
                    hB = B // 2
                    nc.gpsimd.dma_start(cc_in[:, :hB], sumb[:, :hB])
                    nc.gpsimd.dma_start(cc_in[:, hB:], sumb[:, hB:])
                    nc.gpsimd.collective_compute(
                        'AllReduce', OP.add, ins=[cc_in[:].opt()],
                        outs=[cc_out[:].opt()], replica_groups=RG)
                    SB = sp.tile([E, B], b16, tag='SB')
                    nc.gpsimd.dma_start(SB[:, :hB], cc_out[:, :hB])
                    nc.gpsimd.dma_start(SB[:, hB:], cc_out[:, hB:])

                    # curr-node first layer, CU half (AR-independent)
                    pcu = psB.tile([E, B], f32, tag='pB')
                    nc.tensor.matmul(pcu[:], W['w_n1c'], CU[:],
                                     start=True, stop=False)

                    # track-node update (local, AR-independent)
                    sumA_bf = sp.tile([E, AL], b16, tag='sumAb')
                    nc.vector.tensor_copy(sumA_bf[:], sumA[:])
                    pn = psA.tile([E, B], f32, tag='pA')
                    nc.tensor.matmul(pn[:, :AL], W['w_n1c'], TR[:],
                                     start=True, stop=False)
                    nc.tensor.matmul(pn[:, :AL], W['w_n1s'], sumA_bf[:],
                                     start=False, stop=True)
                    htr = sp.tile([E, AL], b16, tag='htr')
                    nc.vector.tensor_scalar(htr[:], pn[:, :AL], Bc['b_n1'], 0.0,
                                            OP.add, OP.max)
                    pn2 = psA.tile([E, B], f32, tag='pA')
                    nc.tensor.matmul(pn2[:, :AL], W['w_n2'], htr[:])
                    TR = sp.tile([E, AL], b16, tag='TR')
                    nc.scalar.activation(TR[:], pn2[:, :AL], AF.Relu, bias=Bc['b_n2'])
                    pn3 = psA.tile([E, B], f32, tag='pA')
                    nc.tensor.matmul(pn3[:, :AL], W['w_ea'], TR[:])
                    abias = sp.tile([E, AL], f32, tag='abias')
                    nc.scalar.activation(abias[:], pn3[:, :AL], AF.Identity,
                                         bias=Bc['b_e1'])

                # deferred classifier tail (overlaps the AllReduce)
                for a in range(min(AL - LAG, AL - DEFER), AL):
                    cls_tile(a)

                # drain this step's classifier outputs
                osl = slice(s * AL * 3, (s + 1) * AL * 3)
                nc.scalar.activation(ostage[:, osl], pc2[:], AF.Copy)
                nc.sync.dma_start(out_d[:, osl], ostage[:, osl])

                if last:
                    break

                # curr-node update (replicated; consumes the AllReduce)
                nc.tensor.matmul(pcu[:], W['w_n1s'], SB[:], start=False, stop=True)
                hcu = sp.tile([E, B], b16, tag='hcu')
                nc.vector.tensor_scalar(hcu[:], pcu[:], Bc['b_n1'], 0.0,
                                        OP.add, OP.max)
                pcu2 = psB.tile([E, B], f32, tag='pB')
                nc.tensor.matmul(pcu2[:], W['w_n2'], hcu[:])
                CU = sp.tile([E, B], b16, tag='CU')
                nc.scalar.activation(CU[:], pcu2[:], AF.Relu, bias=Bc['b_n2'])

    nc.compile()
    return nc


_CACHE = {}


def _get_nc():
    if 'nc' not in _CACHE:
        _CACHE['nc'] = _build_nc()
    return _CACHE['nc']


def _prep_in_maps(inputs):
    f32 = np.float32
    ta = np.asarray(inputs['track_app'], f32)
    ca = np.asarray(inputs['current_app'], f32)
    ta_n = ta / np.linalg.norm(ta, axis=1, keepdims=True)
    ca_n = ca / np.linalg.norm(ca, axis=1, keepdims=True)
    tc_ = np.asarray(inputs['track_coords'], f32)
    cc = np.asarray(inputs['current_coords'], f32)
    tr_x = (tc_[:, 0] + tc_[:, 2]) * .5
    tr_y = (tc_[:, 1] + tc_[:, 3]) * .5
    tr_w = np.abs(tc_[:, 2] - tc_[:, 0])
    tr_h = np.abs(tc_[:, 1] - tc_[:, 3])
    cu_x = (cc[:, 0] + cc[:, 2]) * .5
    cu_y = (cc[:, 1] + cc[:, 3]) * .5
    cu_w = np.abs(cc[:, 2] - cc[:, 0])
    cu_h = np.abs(cc[:, 1] - cc[:, 3])
    den = tr_w[:, None] + cu_w[None, :]
    d_x = 2.0 * (cu_x[None, :] - tr_x[:, None]) / den
    d_y = 2.0 * (cu_y[None, :] - tr_y[:, None]) / den
    d_w = np.log(tr_w[:, None] / cu_w[None, :])
    d_h = np.log(tr_h[:, None] / cu_h[None, :])
    d_t = (np.asarray(inputs['track_t'], f32)[:, None]
           - np.asarray(inputs['curr_t'], f32)[None, :])
    geo = np.stack([d_x, d_y, d_w, d_h, d_t], axis=0)  # (5,A,B)

    W_ei1 = np.asarray(inputs['W_ei1'], f32)
    W_e1 = np.asarray(inputs['W_e1'], f32)
    W_n1 = np.asarray(inputs['W_n1'], f32)

    def col(x):
        return np.ascontiguousarray(np.asarray(x, f32).reshape(E, 1))

    wmats = {
        'w_ei2': np.asarray(inputs['W_ei2'], f32),
        'w_sum': W_e1[0:128] + W_e1[128:256],
        'w_ee': W_e1[0:128],
        'w_eie': W_e1[128:256],
        'w_ea': W_e1[256:384],
        'w_eb': W_e1[384:512],
        'w_e2': np.asarray(inputs['W_e2'], f32),
        'w_c1': np.asarray(inputs['W_c1'], f32),
        'w_n1c': W_n1[:128],
        'w_n1s': W_n1[128:],
        'w_n2': np.asarray(inputs['W_n2'], f32),
    }
    bvecs = {
        'b_cnn': col(inputs['b_cnn']),
        'b_ei1': col(np.asarray(inputs['b_ei1'], f32) + W_ei1[5, :]),
        'b_ei2': col(inputs['b_ei2']),
        'b_e1': col(inputs['b_e1']),
        'b_e2': col(inputs['b_e2']),
        'b_c1': col(inputs['b_c1']),
        'b_n1': col(inputs['b_n1']),
        'b_n2': col(inputs['b_n2']),
    }
    shared = {
        'ca_nt': np.ascontiguousarray(ca_n.T).astype(BF16),
        'ca_rawt': np.ascontiguousarray(ca.T).astype(BF16),
        'w_cnn': np.asarray(inputs['W_cnn'], f32).astype(BF16),
        'w_geo': W_ei1[:5, :].astype(BF16),
        'w_reid': (-W_ei1[5:6, :]).astype(BF16),
        'w_c2': np.asarray(inputs['W_c2'], f32).astype(BF16),
        'wstack': np.concatenate([wmats[n] for n in W_NAMES], axis=1).astype(BF16),
        'bstack': np.concatenate([bvecs[n] for n in B_NAMES], axis=1).astype(f32),
    }
    in_maps = []
    for c in range(NC_N):
        rs = slice(c * AL, (c + 1) * AL)
        m = dict(shared)
        m['geo'] = np.ascontiguousarray(geo[:, rs, :]).reshape(5, AL * B).astype(BF16)
        m['ta_nt'] = np.ascontiguousarray(ta_n[rs].T).astype(BF16)
        m['tr_rawt'] = np.ascontiguousarray(ta[rs].T).astype(BF16)
        in_maps.append(m)
    return in_maps


def _assemble(outs, b_c2):
    # outs: per-core (128, S*AL*3) staging; out[r, s, a, j] = cls[s, a, j*128+r]
    full = []
    for o in outs:
        o = np.asarray(o, np.float32).reshape(128, S, AL, 3)
        full.append(np.transpose(o, (1, 2, 3, 0)).reshape(S, AL, B))
    full = np.concatenate(full, axis=1)  # (S, A, B)
    return (full + np.asarray(b_c2, np.float32)[0]).astype(np.float32)


def kernel(**inputs) -> np.ndarray:
    from concourse.bass_utils import run_bass_kernel_spmd

    in_maps = _prep_in_maps(inputs)
    nc = _get_nc()
    res = run_bass_kernel_spmd(nc, in_maps, core_ids=list(range(NC_N)))
    return _assemble([r['out'] for r in res.results], inputs['b_c2'])


if __name__ == '__main__':
    import reference as R
    inp = {k: np.asarray(v) for k, v in R.setup_inputs().items()}
    out = kernel(**inp)
    exp = np.asarray(R.reference(**inp))
    err = np.linalg.norm(out - exp) / np.linalg.norm(exp)
    print('rel_l2', err)
